# revision 8
# baseline (speedup 1.0000x reference)
"""Trainium2 Bass kernel for nn_EntityLinker (GNN message passing / edge MLP).

Strategy (8 NeuronCores, data-parallel over edges):
  - Each core handles E/8 = 62500 edges; node_repr (cast to bf16 on host) and
    the MLP weights are replicated.
  - h_i / h_j rows are fetched with GpSimd `dma_gather` (transpose mode), which
    lands tiles feature-major ([128 features x T edges]) in SBUF -- exactly the
    layout the tensor engine needs (contraction dim on partitions).
  - dma_gather indices are int16, so node ids >= 32768 cannot be addressed from
    base 0. Edges are bucketed on host into 4 groups by (src>=32768, dst>=32768)
    and each group's gathers use a base pointer into the node table (row 0 or
    row 32768); in-group indices then fit in [0, 32768).
  - Edge MLP runs in bf16 with fp32 PSUM accumulation, feature-major:
      x1^T[256,T] = relu(W1^T @ [hi;hj;|hi-hj|;hi*hj] + b1)
      x2^T[128,T] = relu(W2^T @ x1^T + b2)
      out[T,2]    = x2^T chunks (stationary) @ W3 + b3   (edge-major psum)
  - Outputs accumulate in SBUF and leave with one contiguous DMA per core in a
    [128, S, 2] layout; the host undoes the layout + the bucketing permutation.
"""

import sys

for _p in ("/opt/trn_rl_repo", "/opt/trn_rl_repo/concourse"):
    if _p not in sys.path:
        sys.path.insert(0, _p)

import numpy as np
import ml_dtypes

from concourse import bacc, tile, mybir
from concourse.bass_utils import run_bass_kernel_spmd

BF16 = ml_dtypes.bfloat16
N_CORES = 8
H = 128
T_MM = 512          # edges per matmul tile (max PSUM free dim for f32)
T_GATHER = 2048     # indices per dma_gather instruction (amortizes SWDGE fixed cost)
SPLIT = 32768       # int16-addressable node-table half size


def _build_program(n_tiles_per_group, n_nodes, tot, reps=1):
    """Build the SPMD Bass program for the given per-group tile counts.

    reps > 1 repeats the whole edge loop (identical work, identical result) so
    wall-clock deltas between reps isolate HW execution time from RPC/transfer
    overhead.
    """
    s_tot = tot // H  # number of 128-edge output subtiles

    nc = bacc.Bacc("TRN2", target_bir_lowering=False, debug=False,
                   num_devices=N_CORES)

    node_d = nc.dram_tensor("node", [n_nodes, H], mybir.dt.bfloat16,
                            kind="ExternalInput")
    src_d = nc.dram_tensor("srcidx", [128, tot // 16], mybir.dt.int16,
                           kind="ExternalInput")
    dst_d = nc.dram_tensor("dstidx", [128, tot // 16], mybir.dt.int16,
                           kind="ExternalInput")
    w1_d = nc.dram_tensor("w1", [128, 4, 2 * H], mybir.dt.bfloat16,
                          kind="ExternalInput")
    w2_d = nc.dram_tensor("w2", [128, 2, H], mybir.dt.bfloat16,
                          kind="ExternalInput")
    w3_d = nc.dram_tensor("w3", [128, 2], mybir.dt.bfloat16,
                          kind="ExternalInput")
    b1_d = nc.dram_tensor("b1", [128, 2], mybir.dt.float32,
                          kind="ExternalInput")
    b2_d = nc.dram_tensor("b2", [128, 1], mybir.dt.float32,
                          kind="ExternalInput")
    b3_d = nc.dram_tensor("b3", [128, 4, 2], mybir.dt.float32,
                          kind="ExternalInput")
    out_d = nc.dram_tensor("out", [128, s_tot, 2], mybir.dt.float32,
                           kind="ExternalOutput")

    # per-group gather base views into the node table
    node_lo = node_d[0:min(SPLIT, n_nodes), :]
    node_hi = node_d[SPLIT:n_nodes, :] if n_nodes > SPLIT else node_lo
    g_src_base = [node_lo, node_lo, node_hi, node_hi]
    g_dst_base = [node_lo, node_hi, node_lo, node_hi]

    relu = mybir.ActivationFunctionType.Relu

    with tile.TileContext(nc) as tc:
        with (
            tc.tile_pool(name="const", bufs=1) as cpool,
            tc.tile_pool(name="work", bufs=3) as wpool,
            tc.tile_pool(name="mid", bufs=2) as mpool,
            tc.tile_pool(name="psA", bufs=2, space="PSUM") as psA,
            tc.tile_pool(name="psB", bufs=2, space="PSUM") as psB,
            tc.tile_pool(name="psC", bufs=2, space="PSUM") as psC,
            tc.tile_pool(name="psO", bufs=2, space="PSUM") as psO,
        ):
            src_t = cpool.tile([128, tot // 16], mybir.dt.int16)
            dst_t = cpool.tile([128, tot // 16], mybir.dt.int16)
            w1_t = cpool.tile([128, 4, 2 * H], mybir.dt.bfloat16)
            w2_t = cpool.tile([128, 2, H], mybir.dt.bfloat16)
            w3_t = cpool.tile([128, 2], mybir.dt.bfloat16)
            b1_t = cpool.tile([128, 2], mybir.dt.float32)
            b2_t = cpool.tile([128, 1], mybir.dt.float32)
            b3_t = cpool.tile([128, 4, 2], mybir.dt.float32)
            outb = cpool.tile([128, s_tot, 2], mybir.dt.float32)

            nc.sync.dma_start(src_t[:], src_d[:])
            nc.sync.dma_start(dst_t[:], dst_d[:])
            nc.sync.dma_start(w1_t[:], w1_d[:])
            nc.sync.dma_start(w2_t[:], w2_d[:])
            nc.sync.dma_start(w3_t[:], w3_d[:])
            nc.sync.dma_start(b1_t[:], b1_d[:])
            nc.sync.dma_start(b2_t[:], b2_d[:])
            nc.sync.dma_start(b3_t[:], b3_d[:])

            for _rep in range(reps):
                _emit_edge_loop(nc, tc, n_tiles_per_group, g_src_base,
                                g_dst_base, src_t, dst_t, w1_t, w2_t, w3_t,
                                b1_t, b2_t, b3_t, outb, wpool, mpool,
                                psA, psB, psC, psO, relu)

            nc.sync.dma_start(out_d[:], outb[:])

    nc.compile()
    return nc


def _emit_edge_loop(nc, tc, n_tiles_per_group, g_src_base, g_dst_base,
                    src_t, dst_t, w1_t, w2_t, w3_t, b1_t, b2_t, b3_t, outb,
                    wpool, mpool, psA, psB, psC, psO, relu):
    if True:
        if True:
            pos = 0  # running padded-edge offset (multiple of T_MM)
            for g in range(4):
                n_t = n_tiles_per_group[g]
                if n_t == 0:
                    continue
                n_edges_g = n_t * T_MM
                # gather blocks of up to T_GATHER edges
                blocks = []
                off = 0
                while off < n_edges_g:
                    b = min(T_GATHER, n_edges_g - off)
                    blocks.append((pos + off, b))
                    off += b
                for (bpos, blen) in blocks:
                    hi = wpool.tile([128, 1, T_GATHER], mybir.dt.bfloat16,
                                    tag="hi")
                    hj = wpool.tile([128, 1, T_GATHER], mybir.dt.bfloat16,
                                    tag="hj")
                    i0 = bpos // 16
                    iw = blen // 16
                    nc.gpsimd.dma_gather(
                        hi[:, :, :blen], g_src_base[g], src_t[:, i0:i0 + iw],
                        num_idxs=blen, num_idxs_reg=blen, elem_size=H,
                        transpose=True)
                    nc.gpsimd.dma_gather(
                        hj[:, :, :blen], g_dst_base[g], dst_t[:, i0:i0 + iw],
                        num_idxs=blen, num_idxs_reg=blen, elem_size=H,
                        transpose=True)

                    for tt in range(blen // T_MM):
                        sl = slice(tt * T_MM, (tt + 1) * T_MM)
                        hiv = hi[:, 0, sl]
                        hjv = hj[:, 0, sl]
                        dv = mpool.tile([128, T_MM], mybir.dt.bfloat16,
                                        tag="dv")
                        pv = mpool.tile([128, T_MM], mybir.dt.bfloat16,
                                        tag="pv")
                        nc.vector.tensor_sub(dv[:], hiv, hjv)
                        # |x| on bf16 = clear the sign bit
                        dvu = dv[:].bitcast(mybir.dt.uint16)
                        nc.vector.tensor_scalar(
                            dvu, dvu, 0x7FFF, None,
                            mybir.AluOpType.bitwise_and)
                        nc.vector.tensor_mul(pv[:], hiv, hjv)

                        chunks = (hiv, hjv, dv[:], pv[:])
                        x1a = psA.tile([128, T_MM], mybir.dt.float32,
                                       tag="x1a", space="PSUM")
                        x1b = psB.tile([128, T_MM], mybir.dt.float32,
                                       tag="x1b", space="PSUM")
                        for m, x1p in enumerate((x1a, x1b)):
                            for kc in range(4):
                                nc.tensor.matmul(
                                    x1p[:],
                                    lhsT=w1_t[:, kc, m * H:(m + 1) * H],
                                    rhs=chunks[kc],
                                    start=(kc == 0), stop=(kc == 3))
                        x1s = []
                        for m, x1p in enumerate((x1a, x1b)):
                            x1 = mpool.tile([128, T_MM], mybir.dt.bfloat16,
                                            tag=f"x1_{m}")
                            nc.scalar.activation(x1[:], x1p[:], relu,
                                                 bias=b1_t[:, m:m + 1])
                            x1s.append(x1)

                        x2p = psC.tile([128, T_MM], mybir.dt.float32,
                                       tag="x2p", space="PSUM")
                        for m in range(2):
                            nc.tensor.matmul(x2p[:], lhsT=w2_t[:, m, :],
                                             rhs=x1s[m][:],
                                             start=(m == 0), stop=(m == 1))
                        x2 = mpool.tile([128, T_MM], mybir.dt.bfloat16,
                                        tag="x2")
                        nc.scalar.activation(x2[:], x2p[:], relu,
                                             bias=b2_t[:, 0:1])

                        # matmul PSUM writes must land at 512B-aligned offsets
                        # within a bank, so pad each [128,2] result to a
                        # 128-float row.
                        op = psO.tile([128, 4, 128], mybir.dt.float32,
                                      tag="op", space="PSUM")
                        for j in range(4):
                            nc.tensor.matmul(op[:, j, 0:2],
                                             lhsT=x2[:, j * H:(j + 1) * H],
                                             rhs=w3_t[:],
                                             start=True, stop=True)
                        s0 = (bpos + tt * T_MM) // H
                        nc.vector.tensor_add(outb[:, s0:s0 + 4, :],
                                             op[:, :, 0:2], b3_t[:])
                pos += n_edges_g


def _prep_core(src_c, dst_c, n_tiles_per_group, tot):
    """Bucket one core's edges; return (src16, dst16, slot_to_edge)."""
    bucket = (src_c >= SPLIT).astype(np.int8) * 2 + (dst_c >= SPLIT)
    order = np.argsort(bucket, kind="stable")
    src16 = np.zeros(tot, dtype=np.int16)
    dst16 = np.zeros(tot, dtype=np.int16)
    slot_to_edge = np.full(tot, -1, dtype=np.int64)
    pos = 0
    ostart = 0
    for g in range(4):
        cnt = int((bucket == g).sum())
        sel = order[ostart:ostart + cnt]
        ostart += cnt
        s = src_c[sel]
        d = dst_c[sel]
        if g >= 2:
            s = s - SPLIT
        if g % 2 == 1:
            d = d - SPLIT
        src16[pos:pos + cnt] = s.astype(np.int16)
        dst16[pos:pos + cnt] = d.astype(np.int16)
        slot_to_edge[pos:pos + cnt] = sel
        pos += n_tiles_per_group[g] * T_MM
    return src16, dst16, slot_to_edge


def _wrap_idx(idx16):
    """[TOT] int16 -> [128, TOT/16] wrapped(16) + replicated(x8) layout."""
    w = idx16.reshape(-1, 16).T  # [16, TOT/16]
    return np.ascontiguousarray(np.tile(w, (8, 1)))


_PROGRAM_CACHE = {}


def kernel(node_repr, src, dst, W1, b1, W2, b2, W3, b3, _reps=1):
    node_repr = np.asarray(node_repr, dtype=np.float32)
    src = np.asarray(src).astype(np.int64)
    dst = np.asarray(dst).astype(np.int64)
    W1 = np.asarray(W1, dtype=np.float32)
    b1 = np.asarray(b1, dtype=np.float32)
    W2 = np.asarray(W2, dtype=np.float32)
    b2 = np.asarray(b2, dtype=np.float32)
    W3 = np.asarray(W3, dtype=np.float32)
    b3 = np.asarray(b3, dtype=np.float32)

    n_nodes = node_repr.shape[0]
    E = src.shape[0]
    assert node_repr.shape[1] == H
    assert E % N_CORES == 0
    e_core = E // N_CORES

    # ---- host-side sharding + bucketing ----
    src_pc = src.reshape(N_CORES, e_core)
    dst_pc = dst.reshape(N_CORES, e_core)
    buckets = (src_pc >= SPLIT).astype(np.int8) * 2 + (dst_pc >= SPLIT)
    counts = np.stack([(buckets == g).sum(axis=1) for g in range(4)], axis=1)
    n_tiles_per_group = tuple(
        int(-(-counts[:, g].max() // T_MM)) for g in range(4))
    tot = sum(n_tiles_per_group) * T_MM

    node_bf = node_repr.astype(BF16)
    w1_dev = np.ascontiguousarray(
        W1.astype(BF16).reshape(4, 128, 2 * H).transpose(1, 0, 2))
    w2_dev = np.ascontiguousarray(
        W2.astype(BF16).reshape(2, 128, H).transpose(1, 0, 2))
    w3_dev = np.ascontiguousarray(W3.astype(BF16))
    b1_dev = np.ascontiguousarray(b1.reshape(2, 128).T.astype(np.float32))
    b2_dev = b2.reshape(128, 1).astype(np.float32)
    b3_dev = np.ascontiguousarray(
        np.tile(b3.astype(np.float32), (128, 4)).reshape(128, 4, 2))

    in_maps = []
    slot_maps = []
    for c in range(N_CORES):
        s16, d16, s2e = _prep_core(src_pc[c], dst_pc[c],
                                   n_tiles_per_group, tot)
        slot_maps.append(s2e)
        in_maps.append({
            "node": node_bf,
            "srcidx": _wrap_idx(s16),
            "dstidx": _wrap_idx(d16),
            "w1": w1_dev, "w2": w2_dev, "w3": w3_dev,
            "b1": b1_dev, "b2": b2_dev, "b3": b3_dev,
        })

    # ---- build + compile (cached per tile-count signature) ----
    key = (n_tiles_per_group, n_nodes, tot, _reps)
    nc = _PROGRAM_CACHE.get(key)
    if nc is None:
        nc = _build_program(n_tiles_per_group, n_nodes, tot, reps=_reps)
        _PROGRAM_CACHE[key] = nc

    r = run_bass_kernel_spmd(nc, in_maps, core_ids=list(range(N_CORES)))

    # ---- gather / unpermute ----
    out = np.empty((E, 2), dtype=np.float32)
    for c in range(N_CORES):
        rows = r.results[c]["out"].transpose(1, 0, 2).reshape(-1, 2)
        s2e = slot_maps[c]
        valid = s2e >= 0
        out[c * e_core + s2e[valid]] = rows[valid]

    return out


# revision 16
# speedup vs baseline: 148.1870x; 148.1870x over previous
"""Trainium2 Bass kernel for nn_EntityLinker (GNN message passing / edge MLP).

Strategy (8 NeuronCores, data-parallel over edges):
  - Each core handles E/8 = 62500 edges; node_repr (cast to bf16 on host) and
    the MLP weights are replicated.
  - h_i / h_j rows are fetched with GpSimd `dma_gather` (transpose mode), which
    lands tiles feature-major ([128 features x T edges]) in SBUF -- exactly the
    layout the tensor engine needs (contraction dim on partitions).
  - dma_gather indices are int16, so node ids >= 32768 cannot be addressed from
    base 0. Edges are bucketed on host into 4 groups by (src>=32768, dst>=32768)
    and each group's gathers use a base pointer into the node table (row 0 or
    row 32768); in-group indices then fit in [0, 32768).
  - Edge MLP runs in bf16 with fp32 PSUM accumulation, feature-major:
      x1^T[256,T] = relu(W1^T @ [hi;hj;|hi-hj|;hi*hj] + b1)
      x2^T[128,T] = relu(W2^T @ x1^T + b2)
      out[T,2]    = x2^T chunks (stationary) @ W3 + b3   (edge-major psum)
  - Outputs accumulate in SBUF and leave with one contiguous DMA per core in a
    [128, S, 2] layout; the host undoes the layout + the bucketing permutation.
"""

import sys

for _p in ("/opt/trn_rl_repo", "/opt/trn_rl_repo/concourse"):
    if _p not in sys.path:
        sys.path.insert(0, _p)

import numpy as np
import ml_dtypes

from concourse import bacc, tile, mybir

BF16 = ml_dtypes.bfloat16
N_CORES = 8
H = 128
T_MM = 512          # edges per matmul tile (max PSUM free dim for f32)
T_GATHER = 512      # indices per dma_gather instruction (amortizes SWDGE fixed cost)
SKIP_GATHER = False  # debug: replace gathers with nothing (uses stale tiles)
SKIP_COMPUTE = False  # debug: drop all per-tile compute
SPLIT = 32768       # int16-addressable node-table half size


def _build_program(n_tiles_per_group, n_nodes, tot, reps=1):
    """Build the SPMD Bass program for the given per-group tile counts.

    reps > 1 repeats the whole edge loop (identical work, identical result) so
    wall-clock deltas between reps isolate HW execution time from RPC/transfer
    overhead.
    """
    s_tot = tot // H  # number of 128-edge output subtiles

    nc = bacc.Bacc("TRN2", target_bir_lowering=False, debug=False,
                   num_devices=N_CORES)

    node_d = nc.dram_tensor("node", [n_nodes, H], mybir.dt.bfloat16,
                            kind="ExternalInput")
    src_d = nc.dram_tensor("srcidx", [128, tot // 16], mybir.dt.int16,
                           kind="ExternalInput")
    dst_d = nc.dram_tensor("dstidx", [128, tot // 16], mybir.dt.int16,
                           kind="ExternalInput")
    w1_d = nc.dram_tensor("w1", [128, 4, 2 * H], mybir.dt.bfloat16,
                          kind="ExternalInput")
    w2_d = nc.dram_tensor("w2", [128, 2, H], mybir.dt.bfloat16,
                          kind="ExternalInput")
    w3_d = nc.dram_tensor("w3", [128, 2], mybir.dt.bfloat16,
                          kind="ExternalInput")
    b1_d = nc.dram_tensor("b1", [128, 2], mybir.dt.float32,
                          kind="ExternalInput")
    b2_d = nc.dram_tensor("b2", [128, 1], mybir.dt.float32,
                          kind="ExternalInput")
    b3_d = nc.dram_tensor("b3", [128, 4, 2], mybir.dt.float32,
                          kind="ExternalInput")
    out_d = nc.dram_tensor("out", [128, s_tot, 2], mybir.dt.float32,
                           kind="ExternalOutput")

    # per-group gather base views into the node table
    node_lo = node_d[0:min(SPLIT, n_nodes), :]
    node_hi = node_d[SPLIT:n_nodes, :] if n_nodes > SPLIT else node_lo
    g_src_base = [node_lo, node_lo, node_hi, node_hi]
    g_dst_base = [node_lo, node_hi, node_lo, node_hi]

    relu = mybir.ActivationFunctionType.Relu

    with tile.TileContext(nc) as tc:
        with (
            tc.tile_pool(name="const", bufs=1) as cpool,
            tc.tile_pool(name="work", bufs=3) as wpool,
            tc.tile_pool(name="mid", bufs=2) as mpool,
            tc.tile_pool(name="psA", bufs=2, space="PSUM") as psA,
            tc.tile_pool(name="psB", bufs=2, space="PSUM") as psB,
            tc.tile_pool(name="psC", bufs=2, space="PSUM") as psC,
            tc.tile_pool(name="psO", bufs=2, space="PSUM") as psO,
        ):
            src_t = cpool.tile([128, tot // 16], mybir.dt.int16)
            dst_t = cpool.tile([128, tot // 16], mybir.dt.int16)
            w1_t = cpool.tile([128, 4, 2 * H], mybir.dt.bfloat16)
            w2_t = cpool.tile([128, 2, H], mybir.dt.bfloat16)
            w3_t = cpool.tile([128, 2], mybir.dt.bfloat16)
            b1_t = cpool.tile([128, 2], mybir.dt.float32)
            b2_t = cpool.tile([128, 1], mybir.dt.float32)
            b3_t = cpool.tile([128, 4, 2], mybir.dt.float32)
            outb = cpool.tile([128, s_tot, 2], mybir.dt.float32)

            nc.sync.dma_start(src_t[:], src_d[:])
            nc.sync.dma_start(dst_t[:], dst_d[:])
            nc.sync.dma_start(w1_t[:], w1_d[:])
            nc.sync.dma_start(w2_t[:], w2_d[:])
            nc.sync.dma_start(w3_t[:], w3_d[:])
            nc.sync.dma_start(b1_t[:], b1_d[:])
            nc.sync.dma_start(b2_t[:], b2_d[:])
            nc.sync.dma_start(b3_t[:], b3_d[:])
            if SKIP_COMPUTE:
                nc.vector.memset(outb[:], 0.0)

            for _rep in range(reps):
                _emit_edge_loop(nc, tc, n_tiles_per_group, g_src_base,
                                g_dst_base, src_t, dst_t, w1_t, w2_t, w3_t,
                                b1_t, b2_t, b3_t, outb, wpool, mpool,
                                psA, psB, psC, psO, relu)

            nc.sync.dma_start(out_d[:], outb[:])

    nc.compile()
    return nc


def _emit_edge_loop(nc, tc, n_tiles_per_group, g_src_base, g_dst_base,
                    src_t, dst_t, w1_t, w2_t, w3_t, b1_t, b2_t, b3_t, outb,
                    wpool, mpool, psA, psB, psC, psO, relu):
    if True:
        if True:
            pos = 0  # running padded-edge offset (multiple of T_MM)
            for g in range(4):
                n_t = n_tiles_per_group[g]
                if n_t == 0:
                    continue
                n_edges_g = n_t * T_MM
                # gather blocks of up to T_GATHER edges
                blocks = []
                off = 0
                while off < n_edges_g:
                    b = min(T_GATHER, n_edges_g - off)
                    blocks.append((pos + off, b))
                    off += b
                for (bpos, blen) in blocks:
                    hi = wpool.tile([128, 1, T_GATHER], mybir.dt.bfloat16,
                                    tag="hi")
                    hj = wpool.tile([128, 1, T_GATHER], mybir.dt.bfloat16,
                                    tag="hj")
                    i0 = bpos // 16
                    iw = blen // 16
                    if SKIP_GATHER:
                        nc.gpsimd.memset(hi[:], 0)
                        nc.gpsimd.memset(hj[:], 0)
                    else:
                        nc.gpsimd.dma_gather(
                            hi[:, :, :blen], g_src_base[g],
                            src_t[:, i0:i0 + iw],
                            num_idxs=blen, num_idxs_reg=blen, elem_size=H,
                            transpose=True)
                        nc.gpsimd.dma_gather(
                            hj[:, :, :blen], g_dst_base[g],
                            dst_t[:, i0:i0 + iw],
                            num_idxs=blen, num_idxs_reg=blen, elem_size=H,
                            transpose=True)
                    if SKIP_COMPUTE:
                        continue

                    for tt in range(blen // T_MM):
                        sl = slice(tt * T_MM, (tt + 1) * T_MM)
                        hiv = hi[:, 0, sl]
                        hjv = hj[:, 0, sl]
                        dv = mpool.tile([128, T_MM], mybir.dt.bfloat16,
                                        tag="dv")
                        pv = mpool.tile([128, T_MM], mybir.dt.bfloat16,
                                        tag="pv")
                        nc.vector.tensor_sub(dv[:], hiv, hjv)
                        # |x| on bf16 = clear the sign bit
                        dvu = dv[:].bitcast(mybir.dt.uint16)
                        nc.vector.tensor_scalar(
                            dvu, dvu, 0x7FFF, None,
                            mybir.AluOpType.bitwise_and)
                        nc.vector.tensor_mul(pv[:], hiv, hjv)

                        chunks = (hiv, hjv, dv[:], pv[:])
                        x1a = psA.tile([128, T_MM], mybir.dt.float32,
                                       tag="x1a", space="PSUM")
                        x1b = psB.tile([128, T_MM], mybir.dt.float32,
                                       tag="x1b", space="PSUM")
                        for m, x1p in enumerate((x1a, x1b)):
                            for kc in range(4):
                                nc.tensor.matmul(
                                    x1p[:],
                                    lhsT=w1_t[:, kc, m * H:(m + 1) * H],
                                    rhs=chunks[kc],
                                    start=(kc == 0), stop=(kc == 3))
                        x1s = []
                        for m, x1p in enumerate((x1a, x1b)):
                            x1 = mpool.tile([128, T_MM], mybir.dt.bfloat16,
                                            tag=f"x1_{m}")
                            nc.scalar.activation(x1[:], x1p[:], relu,
                                                 bias=b1_t[:, m:m + 1])
                            x1s.append(x1)

                        x2p = psC.tile([128, T_MM], mybir.dt.float32,
                                       tag="x2p", space="PSUM")
                        for m in range(2):
                            nc.tensor.matmul(x2p[:], lhsT=w2_t[:, m, :],
                                             rhs=x1s[m][:],
                                             start=(m == 0), stop=(m == 1))
                        x2 = mpool.tile([128, T_MM], mybir.dt.bfloat16,
                                        tag="x2")
                        nc.scalar.activation(x2[:], x2p[:], relu,
                                             bias=b2_t[:, 0:1])

                        # matmul PSUM writes must land at 512B-aligned offsets
                        # within a bank, so pad each [128,2] result to a
                        # 128-float row.
                        op = psO.tile([128, 4, 128], mybir.dt.float32,
                                      tag="op", space="PSUM")
                        for j in range(4):
                            nc.tensor.matmul(op[:, j, 0:2],
                                             lhsT=x2[:, j * H:(j + 1) * H],
                                             rhs=w3_t[:],
                                             start=True, stop=True)
                        s0 = (bpos + tt * T_MM) // H
                        nc.vector.tensor_add(outb[:, s0:s0 + 4, :],
                                             op[:, :, 0:2], b3_t[:])
                pos += n_edges_g


def _prep_core(src_c, dst_c, n_tiles_per_group, tot):
    """Bucket one core's edges; return (src16, dst16, slot_to_edge)."""
    bucket = (src_c >= SPLIT).astype(np.int8) * 2 + (dst_c >= SPLIT)
    order = np.argsort(bucket, kind="stable")
    src16 = np.zeros(tot, dtype=np.int16)
    dst16 = np.zeros(tot, dtype=np.int16)
    slot_to_edge = np.full(tot, -1, dtype=np.int64)
    pos = 0
    ostart = 0
    for g in range(4):
        cnt = int((bucket == g).sum())
        sel = order[ostart:ostart + cnt]
        ostart += cnt
        s = src_c[sel]
        d = dst_c[sel]
        if g >= 2:
            s = s - SPLIT
        if g % 2 == 1:
            d = d - SPLIT
        src16[pos:pos + cnt] = s.astype(np.int16)
        dst16[pos:pos + cnt] = d.astype(np.int16)
        slot_to_edge[pos:pos + cnt] = sel
        pos += n_tiles_per_group[g] * T_MM
    return src16, dst16, slot_to_edge


def _wrap_idx(idx16):
    """[TOT] int16 -> [128, TOT/16] wrapped(16) + replicated(x8) layout."""
    w = idx16.reshape(-1, 16).T  # [16, TOT/16]
    return np.ascontiguousarray(np.tile(w, (8, 1)))


_PROGRAM_CACHE = {}


class _Runner:
    """Compile a Bass program once into a reusable sharded PJRT executable.

    Mirrors bass2jax.run_bass_via_pjrt but keeps the jitted callable (and the
    device-resident inputs) alive so repeat invocations only pay dispatch +
    execute, not NEFF re-load.
    """

    def __init__(self, nc):
        import jax
        from jax.sharding import Mesh, PartitionSpec
        from jax.experimental.shard_map import shard_map
        from concourse import bass2jax as b2j

        b2j.install_neuronx_cc_hook()
        self._jax = jax
        self.nc = nc

        in_names, out_names, out_avals, zero_outs = [], [], [], []
        partition_name = (nc.partition_id_tensor.name
                          if nc.partition_id_tensor else None)
        for alloc in nc.m.functions[0].allocations:
            if not isinstance(alloc, mybir.MemoryLocationSet):
                continue
            name = alloc.memorylocations[0].name
            if alloc.kind == "ExternalInput":
                if name != partition_name:
                    in_names.append(name)
            elif alloc.kind == "ExternalOutput":
                out_names.append(name)
                shape = tuple(alloc.tensor_shape)
                dtype = mybir.dt.np(alloc.dtype)
                out_avals.append(jax.core.ShapedArray(shape, dtype))
                zero_outs.append(np.zeros(shape, dtype))
        self.in_names = in_names
        self.out_names = out_names
        self.out_avals = out_avals
        self.zero_outs = zero_outs
        n_params = len(in_names)
        n_outs = len(out_names)
        all_in_names = list(in_names) + list(out_names)
        if partition_name is not None:
            all_in_names.append(partition_name)

        def _body(*args):
            operands = list(args)
            if partition_name is not None:
                operands.append(b2j.partition_id_tensor())
            outs = b2j._bass_exec_p.bind(
                *operands,
                out_avals=tuple(out_avals),
                in_names=tuple(all_in_names),
                out_names=tuple(out_names),
                lowering_input_output_aliases=(),
                sim_require_finite=True,
                sim_require_nnan=True,
                nc=nc,
            )
            return tuple(outs)

        devices = jax.devices()[:N_CORES]
        mesh = Mesh(np.asarray(devices), ("core",))
        in_specs = (PartitionSpec("core"),) * (n_params + n_outs)
        out_specs = (PartitionSpec("core"),) * n_outs
        donate = tuple(range(n_params, n_params + n_outs))
        self.sharded = jax.jit(
            shard_map(_body, mesh=mesh, in_specs=in_specs,
                      out_specs=out_specs, check_rep=False),
            donate_argnums=donate, keep_unused=True)
        self._resident = None

    def set_inputs(self, in_maps):
        jax = self._jax
        concat_in = [
            np.concatenate([np.asarray(in_maps[c][n])
                            for c in range(N_CORES)], axis=0)
            for n in self.in_names
        ]
        self._resident = [jax.device_put(a) for a in concat_in]
        jax.block_until_ready(self._resident)

    def execute(self):
        jax = self._jax
        zeros = [np.zeros((N_CORES * z.shape[0], *z.shape[1:]), z.dtype)
                 for z in self.zero_outs]
        out_arrs = self.sharded(*self._resident, *zeros)
        jax.block_until_ready(out_arrs)
        return out_arrs

    def run(self):
        out_arrs = self.execute()
        return [
            {name: np.asarray(out_arrs[i]).reshape(
                N_CORES, *self.out_avals[i].shape)[c]
             for i, name in enumerate(self.out_names)}
            for c in range(N_CORES)
        ]


def kernel(node_repr, src, dst, W1, b1, W2, b2, W3, b3, _reps=1):
    node_repr = np.asarray(node_repr, dtype=np.float32)
    src = np.asarray(src).astype(np.int64)
    dst = np.asarray(dst).astype(np.int64)
    W1 = np.asarray(W1, dtype=np.float32)
    b1 = np.asarray(b1, dtype=np.float32)
    W2 = np.asarray(W2, dtype=np.float32)
    b2 = np.asarray(b2, dtype=np.float32)
    W3 = np.asarray(W3, dtype=np.float32)
    b3 = np.asarray(b3, dtype=np.float32)

    n_nodes = node_repr.shape[0]
    E = src.shape[0]
    assert node_repr.shape[1] == H
    assert E % N_CORES == 0
    e_core = E // N_CORES

    # ---- host-side sharding + bucketing ----
    src_pc = src.reshape(N_CORES, e_core)
    dst_pc = dst.reshape(N_CORES, e_core)
    buckets = (src_pc >= SPLIT).astype(np.int8) * 2 + (dst_pc >= SPLIT)
    counts = np.stack([(buckets == g).sum(axis=1) for g in range(4)], axis=1)
    n_tiles_per_group = tuple(
        int(-(-counts[:, g].max() // T_MM)) for g in range(4))
    tot = sum(n_tiles_per_group) * T_MM

    node_bf = node_repr.astype(BF16)
    w1_dev = np.ascontiguousarray(
        W1.astype(BF16).reshape(4, 128, 2 * H).transpose(1, 0, 2))
    w2_dev = np.ascontiguousarray(
        W2.astype(BF16).reshape(2, 128, H).transpose(1, 0, 2))
    w3_dev = np.ascontiguousarray(W3.astype(BF16))
    b1_dev = np.ascontiguousarray(b1.reshape(2, 128).T.astype(np.float32))
    b2_dev = b2.reshape(128, 1).astype(np.float32)
    b3_dev = np.ascontiguousarray(
        np.tile(b3.astype(np.float32), (128, 4)).reshape(128, 4, 2))

    in_maps = []
    slot_maps = []
    for c in range(N_CORES):
        s16, d16, s2e = _prep_core(src_pc[c], dst_pc[c],
                                   n_tiles_per_group, tot)
        slot_maps.append(s2e)
        in_maps.append({
            "node": node_bf,
            "srcidx": _wrap_idx(s16),
            "dstidx": _wrap_idx(d16),
            "w1": w1_dev, "w2": w2_dev, "w3": w3_dev,
            "b1": b1_dev, "b2": b2_dev, "b3": b3_dev,
        })

    # ---- build + compile (cached per tile-count signature) ----
    key = (n_tiles_per_group, n_nodes, tot, _reps)
    runner = _PROGRAM_CACHE.get(key)
    if runner is None:
        nc = _build_program(n_tiles_per_group, n_nodes, tot, reps=_reps)
        runner = _Runner(nc)
        _PROGRAM_CACHE[key] = runner
    runner.set_inputs(in_maps)
    results = runner.run()

    # ---- gather / unpermute ----
    out = np.empty((E, 2), dtype=np.float32)
    for c in range(N_CORES):
        rows = results[c]["out"].transpose(1, 0, 2).reshape(-1, 2)
        s2e = slot_maps[c]
        valid = s2e >= 0
        out[c * e_core + s2e[valid]] = rows[valid]

    global _LAST_RUNNER
    _LAST_RUNNER = runner
    return out


_LAST_RUNNER = None


# revision 22
# speedup vs baseline: 172.4415x; 1.1637x over previous
"""Trainium2 Bass kernel for nn_EntityLinker (GNN message passing / edge MLP).

Strategy (8 NeuronCores, data-parallel over edges):
  - Each core handles E/8 = 62500 edges; node_repr (cast to bf16 on host) and
    the MLP weights are replicated.
  - h_i / h_j rows are fetched with GpSimd `dma_gather` (transpose mode), which
    lands tiles feature-major ([128 features x T edges]) in SBUF -- exactly the
    layout the tensor engine needs (contraction dim on partitions).
  - dma_gather indices are int16, so node ids >= 32768 cannot be addressed from
    base 0. Edges are bucketed on host into 4 groups by (src>=32768, dst>=32768)
    and each group's gathers use a base pointer into the node table (row 0 or
    row 32768); in-group indices then fit in [0, 32768).
  - Edge MLP runs in bf16 with fp32 PSUM accumulation, feature-major:
      x1^T[256,T] = relu(W1^T @ [hi;hj;|hi-hj|;hi*hj] + b1)
      x2^T[128,T] = relu(W2^T @ x1^T + b2)
      out[T,2]    = x2^T chunks (stationary) @ W3 + b3   (edge-major psum)
  - Outputs accumulate in SBUF and leave with one contiguous DMA per core in a
    [128, S, 2] layout; the host undoes the layout + the bucketing permutation.
"""

import sys

for _p in ("/opt/trn_rl_repo", "/opt/trn_rl_repo/concourse"):
    if _p not in sys.path:
        sys.path.insert(0, _p)

import numpy as np
import ml_dtypes

from concourse import bacc, tile, mybir

BF16 = ml_dtypes.bfloat16
N_CORES = 8
H = 128
T_MM = 512          # edges per matmul tile (max PSUM free dim for f32)
T_GATHER = 512      # indices per dma_gather instruction (amortizes SWDGE fixed cost)
DMA_SCRATCH = 16384  # SWDGE descriptor-ring carveout (bytes/partition)
N_QUEUES = 1        # SWDGE queues (src/dst gathers split across them if 2)
SBUF_NODE = False   # keep the node table SBUF-resident and gather from SBUF
SKIP_GATHER = False  # debug: replace gathers with nothing (uses stale tiles)
SKIP_COMPUTE = False  # debug: drop all per-tile compute
SPLIT = 32768       # int16-addressable node-table half size


def _build_program(n_tiles_per_group, n_nodes, tot, reps=1):
    """Build the SPMD Bass program for the given per-group tile counts.

    reps > 1 repeats the whole edge loop (identical work, identical result) so
    wall-clock deltas between reps isolate HW execution time from RPC/transfer
    overhead.
    """
    s_tot = tot // H  # number of 128-edge output subtiles

    nc = bacc.Bacc("TRN2", target_bir_lowering=False, debug=False,
                   num_devices=N_CORES,
                   dynamic_dma_scratch_size=DMA_SCRATCH,
                   num_swdge_queues=N_QUEUES)

    if SBUF_NODE:
        # node table pre-arranged on host as [128, n_ranks*H] bf16:
        # node row (rank*128 + tok) lives at partition tok, byte range
        # [rank*256, rank*256+256).
        n_ranks = -(-n_nodes // 128)
        node_d = nc.dram_tensor("node", [128, n_ranks * H], mybir.dt.bfloat16,
                                kind="ExternalInput")
    else:
        node_d = nc.dram_tensor("node", [n_nodes, H], mybir.dt.bfloat16,
                                kind="ExternalInput")
    src_d = nc.dram_tensor("srcidx", [128, tot // 16], mybir.dt.int16,
                           kind="ExternalInput")
    dst_d = nc.dram_tensor("dstidx", [128, tot // 16], mybir.dt.int16,
                           kind="ExternalInput")
    w1_d = nc.dram_tensor("w1", [128, 4, 2 * H], mybir.dt.bfloat16,
                          kind="ExternalInput")
    w2_d = nc.dram_tensor("w2", [128, 2, H], mybir.dt.bfloat16,
                          kind="ExternalInput")
    w3_d = nc.dram_tensor("w3", [128, 2], mybir.dt.bfloat16,
                          kind="ExternalInput")
    b1_d = nc.dram_tensor("b1", [128, 2], mybir.dt.float32,
                          kind="ExternalInput")
    b2_d = nc.dram_tensor("b2", [128, 1], mybir.dt.float32,
                          kind="ExternalInput")
    b3_d = nc.dram_tensor("b3", [128, 4, 2], mybir.dt.float32,
                          kind="ExternalInput")
    out_d = nc.dram_tensor("out", [128, s_tot, 2], mybir.dt.float32,
                           kind="ExternalOutput")

    relu = mybir.ActivationFunctionType.Relu

    with tile.TileContext(nc) as tc:
        with (
            tc.tile_pool(name="const", bufs=1) as cpool,
            tc.tile_pool(name="work", bufs=3) as wpool,
            tc.tile_pool(name="mid", bufs=2) as mpool,
            tc.tile_pool(name="psA", bufs=2, space="PSUM") as psA,
            tc.tile_pool(name="psB", bufs=2, space="PSUM") as psB,
            tc.tile_pool(name="psC", bufs=2, space="PSUM") as psC,
            tc.tile_pool(name="psO", bufs=2, space="PSUM") as psO,
        ):
            src_t = cpool.tile([128, tot // 16], mybir.dt.int16)
            dst_t = cpool.tile([128, tot // 16], mybir.dt.int16)
            w1_t = cpool.tile([128, 4, 2 * H], mybir.dt.bfloat16)
            w2_t = cpool.tile([128, 2, H], mybir.dt.bfloat16)
            w3_t = cpool.tile([128, 2], mybir.dt.bfloat16)
            b1_t = cpool.tile([128, 2], mybir.dt.float32)
            b2_t = cpool.tile([128, 1], mybir.dt.float32)
            b3_t = cpool.tile([128, 4, 2], mybir.dt.float32)
            outb = cpool.tile([128, s_tot, 2], mybir.dt.float32)

            nc.sync.dma_start(src_t[:], src_d[:])
            nc.sync.dma_start(dst_t[:], dst_d[:])
            nc.sync.dma_start(w1_t[:], w1_d[:])
            nc.sync.dma_start(w2_t[:], w2_d[:])
            nc.sync.dma_start(w3_t[:], w3_d[:])
            nc.sync.dma_start(b1_t[:], b1_d[:])
            nc.sync.dma_start(b2_t[:], b2_d[:])
            nc.sync.dma_start(b3_t[:], b3_d[:])
            if SKIP_COMPUTE:
                nc.vector.memset(outb[:], 0.0)

            if SBUF_NODE:
                n_ranks = -(-n_nodes // 128)
                node_t = cpool.tile([128, n_ranks * H], mybir.dt.bfloat16)
                nc.sync.dma_start(node_t[:], node_d[:])
                node_lo = node_t[:, 0:min(SPLIT, n_ranks * H)]
                node_hi = (node_t[:, SPLIT:n_ranks * H]
                           if n_ranks * H > SPLIT else node_lo)
            else:
                node_lo = node_d[0:min(SPLIT, n_nodes), :]
                node_hi = (node_d[SPLIT:n_nodes, :]
                           if n_nodes > SPLIT else node_lo)
            g_src_base = [node_lo, node_lo, node_hi, node_hi]
            g_dst_base = [node_lo, node_hi, node_lo, node_hi]

            for _rep in range(reps):
                _emit_edge_loop(nc, tc, n_tiles_per_group, g_src_base,
                                g_dst_base, src_t, dst_t, w1_t, w2_t, w3_t,
                                b1_t, b2_t, b3_t, outb, wpool, mpool,
                                psA, psB, psC, psO, relu)

            nc.sync.dma_start(out_d[:], outb[:])

    nc.compile()
    return nc


def _emit_edge_loop(nc, tc, n_tiles_per_group, g_src_base, g_dst_base,
                    src_t, dst_t, w1_t, w2_t, w3_t, b1_t, b2_t, b3_t, outb,
                    wpool, mpool, psA, psB, psC, psO, relu):
    if True:
        if True:
            pos = 0  # running padded-edge offset (multiple of T_MM)
            for g in range(4):
                n_t = n_tiles_per_group[g]
                if n_t == 0:
                    continue
                n_edges_g = n_t * T_MM
                # gather blocks of up to T_GATHER edges
                blocks = []
                off = 0
                while off < n_edges_g:
                    b = min(T_GATHER, n_edges_g - off)
                    blocks.append((pos + off, b))
                    off += b
                for (bpos, blen) in blocks:
                    hi = wpool.tile([128, 1, T_GATHER], mybir.dt.bfloat16,
                                    tag="hi")
                    hj = wpool.tile([128, 1, T_GATHER], mybir.dt.bfloat16,
                                    tag="hj")
                    i0 = bpos // 16
                    iw = blen // 16
                    gkw = dict(num_idxs=blen, num_idxs_reg=blen, elem_size=H,
                               transpose=True)
                    if SBUF_NODE:
                        gkw.update(sbuf_tokens_per_rank=128,
                                   sbuf_free_dim_per_rank=2 * H,
                                   sbuf_free_dim_pad_per_rank=0,
                                   sbuf_byte_offset=0)
                    if SKIP_GATHER:
                        nc.gpsimd.memset(hi[:], 0)
                        nc.gpsimd.memset(hj[:], 0)
                    else:
                        nc.gpsimd.dma_gather(
                            hi[:, :, :blen], g_src_base[g],
                            src_t[:, i0:i0 + iw], queue_num=0, **gkw)
                        nc.gpsimd.dma_gather(
                            hj[:, :, :blen], g_dst_base[g],
                            dst_t[:, i0:i0 + iw],
                            queue_num=(1 if N_QUEUES > 1 else 0), **gkw)
                    if SKIP_COMPUTE:
                        continue

                    for tt in range(blen // T_MM):
                        sl = slice(tt * T_MM, (tt + 1) * T_MM)
                        hiv = hi[:, 0, sl]
                        hjv = hj[:, 0, sl]
                        dv = mpool.tile([128, T_MM], mybir.dt.bfloat16,
                                        tag="dv")
                        pv = mpool.tile([128, T_MM], mybir.dt.bfloat16,
                                        tag="pv")
                        nc.vector.tensor_sub(dv[:], hiv, hjv)
                        # |x| on bf16 = clear the sign bit
                        dvu = dv[:].bitcast(mybir.dt.uint16)
                        nc.vector.tensor_scalar(
                            dvu, dvu, 0x7FFF, None,
                            mybir.AluOpType.bitwise_and)
                        nc.vector.tensor_mul(pv[:], hiv, hjv)

                        chunks = (hiv, hjv, dv[:], pv[:])
                        x1a = psA.tile([128, T_MM], mybir.dt.float32,
                                       tag="x1a", space="PSUM")
                        x1b = psB.tile([128, T_MM], mybir.dt.float32,
                                       tag="x1b", space="PSUM")
                        for m, x1p in enumerate((x1a, x1b)):
                            for kc in range(4):
                                nc.tensor.matmul(
                                    x1p[:],
                                    lhsT=w1_t[:, kc, m * H:(m + 1) * H],
                                    rhs=chunks[kc],
                                    start=(kc == 0), stop=(kc == 3))
                        x1s = []
                        for m, x1p in enumerate((x1a, x1b)):
                            x1 = mpool.tile([128, T_MM], mybir.dt.bfloat16,
                                            tag=f"x1_{m}")
                            nc.scalar.activation(x1[:], x1p[:], relu,
                                                 bias=b1_t[:, m:m + 1])
                            x1s.append(x1)

                        x2p = psC.tile([128, T_MM], mybir.dt.float32,
                                       tag="x2p", space="PSUM")
                        for m in range(2):
                            nc.tensor.matmul(x2p[:], lhsT=w2_t[:, m, :],
                                             rhs=x1s[m][:],
                                             start=(m == 0), stop=(m == 1))
                        x2 = mpool.tile([128, T_MM], mybir.dt.bfloat16,
                                        tag="x2")
                        nc.scalar.activation(x2[:], x2p[:], relu,
                                             bias=b2_t[:, 0:1])

                        # matmul PSUM writes must land at 512B-aligned offsets
                        # within a bank, so pad each [128,2] result to a
                        # 128-float row.
                        op = psO.tile([128, 4, 128], mybir.dt.float32,
                                      tag="op", space="PSUM")
                        for j in range(4):
                            nc.tensor.matmul(op[:, j, 0:2],
                                             lhsT=x2[:, j * H:(j + 1) * H],
                                             rhs=w3_t[:],
                                             start=True, stop=True)
                        s0 = (bpos + tt * T_MM) // H
                        nc.vector.tensor_add(outb[:, s0:s0 + 4, :],
                                             op[:, :, 0:2], b3_t[:])
                pos += n_edges_g


def _prep_core(src_c, dst_c, n_tiles_per_group, tot):
    """Bucket one core's edges; return (src16, dst16, slot_to_edge)."""
    bucket = (src_c >= SPLIT).astype(np.int8) * 2 + (dst_c >= SPLIT)
    order = np.argsort(bucket, kind="stable")
    src16 = np.zeros(tot, dtype=np.int16)
    dst16 = np.zeros(tot, dtype=np.int16)
    slot_to_edge = np.full(tot, -1, dtype=np.int64)
    pos = 0
    ostart = 0
    for g in range(4):
        cnt = int((bucket == g).sum())
        sel = order[ostart:ostart + cnt]
        ostart += cnt
        s = src_c[sel]
        d = dst_c[sel]
        if g >= 2:
            s = s - SPLIT
        if g % 2 == 1:
            d = d - SPLIT
        src16[pos:pos + cnt] = s.astype(np.int16)
        dst16[pos:pos + cnt] = d.astype(np.int16)
        slot_to_edge[pos:pos + cnt] = sel
        pos += n_tiles_per_group[g] * T_MM
    return src16, dst16, slot_to_edge


def _wrap_idx(idx16):
    """[TOT] int16 -> [128, TOT/16] wrapped(16) + replicated(x8) layout."""
    w = idx16.reshape(-1, 16).T  # [16, TOT/16]
    return np.ascontiguousarray(np.tile(w, (8, 1)))


_PROGRAM_CACHE = {}


class _Runner:
    """Compile a Bass program once into a reusable sharded PJRT executable.

    Mirrors bass2jax.run_bass_via_pjrt but keeps the jitted callable (and the
    device-resident inputs) alive so repeat invocations only pay dispatch +
    execute, not NEFF re-load.
    """

    def __init__(self, nc):
        import jax
        from jax.sharding import Mesh, PartitionSpec
        from jax.experimental.shard_map import shard_map
        from concourse import bass2jax as b2j

        b2j.install_neuronx_cc_hook()
        self._jax = jax
        self.nc = nc

        in_names, out_names, out_avals, zero_outs = [], [], [], []
        partition_name = (nc.partition_id_tensor.name
                          if nc.partition_id_tensor else None)
        for alloc in nc.m.functions[0].allocations:
            if not isinstance(alloc, mybir.MemoryLocationSet):
                continue
            name = alloc.memorylocations[0].name
            if alloc.kind == "ExternalInput":
                if name != partition_name:
                    in_names.append(name)
            elif alloc.kind == "ExternalOutput":
                out_names.append(name)
                shape = tuple(alloc.tensor_shape)
                dtype = mybir.dt.np(alloc.dtype)
                out_avals.append(jax.core.ShapedArray(shape, dtype))
                zero_outs.append(np.zeros(shape, dtype))
        self.in_names = in_names
        self.out_names = out_names
        self.out_avals = out_avals
        self.zero_outs = zero_outs
        n_params = len(in_names)
        n_outs = len(out_names)
        all_in_names = list(in_names) + list(out_names)
        if partition_name is not None:
            all_in_names.append(partition_name)

        def _body(*args):
            operands = list(args)
            if partition_name is not None:
                operands.append(b2j.partition_id_tensor())
            outs = b2j._bass_exec_p.bind(
                *operands,
                out_avals=tuple(out_avals),
                in_names=tuple(all_in_names),
                out_names=tuple(out_names),
                lowering_input_output_aliases=(),
                sim_require_finite=True,
                sim_require_nnan=True,
                nc=nc,
            )
            return tuple(outs)

        devices = jax.devices()[:N_CORES]
        mesh = Mesh(np.asarray(devices), ("core",))
        in_specs = (PartitionSpec("core"),) * (n_params + n_outs)
        out_specs = (PartitionSpec("core"),) * n_outs
        donate = tuple(range(n_params, n_params + n_outs))
        self.sharded = jax.jit(
            shard_map(_body, mesh=mesh, in_specs=in_specs,
                      out_specs=out_specs, check_rep=False),
            donate_argnums=donate, keep_unused=True)
        self._resident = None

    def set_inputs(self, in_maps):
        jax = self._jax
        concat_in = [
            np.concatenate([np.asarray(in_maps[c][n])
                            for c in range(N_CORES)], axis=0)
            for n in self.in_names
        ]
        self._resident = [jax.device_put(a) for a in concat_in]
        jax.block_until_ready(self._resident)

    def execute(self):
        jax = self._jax
        zeros = [np.zeros((N_CORES * z.shape[0], *z.shape[1:]), z.dtype)
                 for z in self.zero_outs]
        out_arrs = self.sharded(*self._resident, *zeros)
        jax.block_until_ready(out_arrs)
        return out_arrs

    def run(self):
        out_arrs = self.execute()
        return [
            {name: np.asarray(out_arrs[i]).reshape(
                N_CORES, *self.out_avals[i].shape)[c]
             for i, name in enumerate(self.out_names)}
            for c in range(N_CORES)
        ]


def kernel(node_repr, src, dst, W1, b1, W2, b2, W3, b3, _reps=1):
    node_repr = np.asarray(node_repr, dtype=np.float32)
    src = np.asarray(src).astype(np.int64)
    dst = np.asarray(dst).astype(np.int64)
    W1 = np.asarray(W1, dtype=np.float32)
    b1 = np.asarray(b1, dtype=np.float32)
    W2 = np.asarray(W2, dtype=np.float32)
    b2 = np.asarray(b2, dtype=np.float32)
    W3 = np.asarray(W3, dtype=np.float32)
    b3 = np.asarray(b3, dtype=np.float32)

    n_nodes = node_repr.shape[0]
    E = src.shape[0]
    assert node_repr.shape[1] == H
    assert E % N_CORES == 0
    e_core = E // N_CORES

    # ---- host-side sharding + bucketing ----
    src_pc = src.reshape(N_CORES, e_core)
    dst_pc = dst.reshape(N_CORES, e_core)
    buckets = (src_pc >= SPLIT).astype(np.int8) * 2 + (dst_pc >= SPLIT)
    counts = np.stack([(buckets == g).sum(axis=1) for g in range(4)], axis=1)
    n_tiles_per_group = tuple(
        int(-(-counts[:, g].max() // T_MM)) for g in range(4))
    tot = sum(n_tiles_per_group) * T_MM

    node_bf = node_repr.astype(BF16)
    if SBUF_NODE:
        n_ranks = -(-n_nodes // 128)
        pad = np.zeros((n_ranks * 128, H), dtype=BF16)
        pad[:n_nodes] = node_bf
        node_bf = np.ascontiguousarray(
            pad.reshape(n_ranks, 128, H).transpose(1, 0, 2)
            .reshape(128, n_ranks * H))
    w1_dev = np.ascontiguousarray(
        W1.astype(BF16).reshape(4, 128, 2 * H).transpose(1, 0, 2))
    w2_dev = np.ascontiguousarray(
        W2.astype(BF16).reshape(2, 128, H).transpose(1, 0, 2))
    w3_dev = np.ascontiguousarray(W3.astype(BF16))
    b1_dev = np.ascontiguousarray(b1.reshape(2, 128).T.astype(np.float32))
    b2_dev = b2.reshape(128, 1).astype(np.float32)
    b3_dev = np.ascontiguousarray(
        np.tile(b3.astype(np.float32), (128, 4)).reshape(128, 4, 2))

    in_maps = []
    slot_maps = []
    for c in range(N_CORES):
        s16, d16, s2e = _prep_core(src_pc[c], dst_pc[c],
                                   n_tiles_per_group, tot)
        slot_maps.append(s2e)
        in_maps.append({
            "node": node_bf,
            "srcidx": _wrap_idx(s16),
            "dstidx": _wrap_idx(d16),
            "w1": w1_dev, "w2": w2_dev, "w3": w3_dev,
            "b1": b1_dev, "b2": b2_dev, "b3": b3_dev,
        })

    # ---- build + compile (cached per tile-count signature) ----
    key = (n_tiles_per_group, n_nodes, tot, _reps)
    runner = _PROGRAM_CACHE.get(key)
    if runner is None:
        nc = _build_program(n_tiles_per_group, n_nodes, tot, reps=_reps)
        runner = _Runner(nc)
        _PROGRAM_CACHE[key] = runner
    runner.set_inputs(in_maps)
    results = runner.run()

    # ---- gather / unpermute ----
    out = np.empty((E, 2), dtype=np.float32)
    for c in range(N_CORES):
        rows = results[c]["out"].transpose(1, 0, 2).reshape(-1, 2)
        s2e = slot_maps[c]
        valid = s2e >= 0
        out[c * e_core + s2e[valid]] = rows[valid]

    global _LAST_RUNNER
    _LAST_RUNNER = runner
    return out


_LAST_RUNNER = None


# revision 35
# speedup vs baseline: 596.6029x; 3.4597x over previous
"""Trainium2 Bass kernel for nn_EntityLinker (GNN message passing / edge MLP).

Strategy (8 NeuronCores, data-parallel over edges):
  - Each core handles E/8 = 62500 edges; node_repr (cast to bf16 on host) and
    the MLP weights are replicated.
  - h_i / h_j rows are fetched with GpSimd `dma_gather` (transpose mode), which
    lands tiles feature-major ([128 features x T edges]) in SBUF -- exactly the
    layout the tensor engine needs (contraction dim on partitions).
  - dma_gather indices are int16, so node ids >= 32768 cannot be addressed from
    base 0. Edges are bucketed on host into 4 groups by (src>=32768, dst>=32768)
    and each group's gathers use a base pointer into the node table (row 0 or
    row 32768); in-group indices then fit in [0, 32768).
  - Edge MLP runs in bf16 with fp32 PSUM accumulation, feature-major:
      x1^T[256,T] = relu(W1^T @ [hi;hj;|hi-hj|;hi*hj] + b1)
      x2^T[128,T] = relu(W2^T @ x1^T + b2)
      out[T,2]    = x2^T chunks (stationary) @ W3 + b3   (edge-major psum)
  - Outputs accumulate in SBUF and leave with one contiguous DMA per core in a
    [128, S, 2] layout; the host undoes the layout + the bucketing permutation.
"""

import sys

for _p in ("/opt/trn_rl_repo", "/opt/trn_rl_repo/concourse"):
    if _p not in sys.path:
        sys.path.insert(0, _p)

import numpy as np
import ml_dtypes

from concourse import bacc, tile, mybir

BF16 = ml_dtypes.bfloat16
N_CORES = 8
H = 128
T_MM = 512          # edges per matmul tile (max PSUM free dim for f32)
T_GATHER = 512      # indices per dma_gather instruction (ring caps transpose
                    # gathers at ~512 descriptors; must be a multiple of T_MM)
DMA_SCRATCH = 16384  # SWDGE descriptor-ring carveout (fixed by ucode)
N_QUEUES = 2        # SWDGE queues; src/dst gathers alternate across them
SBUF_NODE = False   # keep the node table SBUF-resident and gather from SBUF
WBUFS = 8           # gather-tile double-buffering depth
MBUFS = 4           # mid-tile (dv/pv/x1/x2) buffering depth
NT_GATHER = False   # debug: non-transpose gathers (edge-major layout, HBM src)
SORT_SRC = True     # sort edges by src within buckets (HBM row locality)
ZERO_IDX = False    # debug: gather with all indices = 0 (locality probe)
SKIP_GATHER = False  # debug: replace gathers with nothing (uses stale tiles)
SKIP_COMPUTE = False  # debug: drop all per-tile compute
SPLIT = 32768       # int16-addressable node-table half size


def _build_program(n_tiles_per_group, n_nodes, tot, reps=1):
    """Build the SPMD Bass program for the given per-group tile counts.

    reps > 1 repeats the whole edge loop (identical work, identical result) so
    wall-clock deltas between reps isolate HW execution time from RPC/transfer
    overhead.
    """
    s_tot = tot // H  # number of 128-edge output subtiles

    nc = bacc.Bacc("TRN2", target_bir_lowering=False, debug=False,
                   num_devices=N_CORES,
                   dynamic_dma_scratch_size=DMA_SCRATCH,
                   num_swdge_queues=N_QUEUES)

    if SBUF_NODE:
        # node table pre-arranged on host as [128, n_ranks*H] bf16:
        # node row (rank*128 + tok) lives at partition tok, byte range
        # [rank*256, rank*256+256).
        n_ranks = -(-n_nodes // 128)
        node_d = nc.dram_tensor("node", [128, n_ranks * H], mybir.dt.bfloat16,
                                kind="ExternalInput")
    else:
        node_d = nc.dram_tensor("node", [n_nodes, H], mybir.dt.bfloat16,
                                kind="ExternalInput")
    src_d = nc.dram_tensor("srcidx", [128, tot // 16], mybir.dt.int16,
                           kind="ExternalInput")
    dst_d = nc.dram_tensor("dstidx", [128, tot // 16], mybir.dt.int16,
                           kind="ExternalInput")
    w1_d = nc.dram_tensor("w1", [128, 4, 2 * H], mybir.dt.bfloat16,
                          kind="ExternalInput")
    w2_d = nc.dram_tensor("w2", [128, 2, H], mybir.dt.bfloat16,
                          kind="ExternalInput")
    w3_d = nc.dram_tensor("w3", [128, 2], mybir.dt.bfloat16,
                          kind="ExternalInput")
    b1_d = nc.dram_tensor("b1", [128, 2], mybir.dt.float32,
                          kind="ExternalInput")
    b2_d = nc.dram_tensor("b2", [128, 1], mybir.dt.float32,
                          kind="ExternalInput")
    b3_d = nc.dram_tensor("b3", [128, 4, 2], mybir.dt.float32,
                          kind="ExternalInput")
    out_d = nc.dram_tensor("out", [128, s_tot, 2], mybir.dt.float32,
                           kind="ExternalOutput")

    relu = mybir.ActivationFunctionType.Relu

    with tile.TileContext(nc) as tc:
        with (
            tc.tile_pool(name="const", bufs=1) as cpool,
            tc.tile_pool(name="work", bufs=WBUFS) as wpool,
            tc.tile_pool(name="mid", bufs=MBUFS) as mpool,
            tc.tile_pool(name="psA", bufs=2, space="PSUM") as psA,
            tc.tile_pool(name="psB", bufs=2, space="PSUM") as psB,
            tc.tile_pool(name="psC", bufs=2, space="PSUM") as psC,
            tc.tile_pool(name="psO", bufs=2, space="PSUM") as psO,
        ):
            src_t = cpool.tile([128, tot // 16], mybir.dt.int16)
            dst_t = cpool.tile([128, tot // 16], mybir.dt.int16)
            w1_t = cpool.tile([128, 4, 2 * H], mybir.dt.bfloat16)
            w2_t = cpool.tile([128, 2, H], mybir.dt.bfloat16)
            w3_t = cpool.tile([128, 2], mybir.dt.bfloat16)
            b1_t = cpool.tile([128, 2], mybir.dt.float32)
            b2_t = cpool.tile([128, 1], mybir.dt.float32)
            b3_t = cpool.tile([128, 4, 2], mybir.dt.float32)
            outb = cpool.tile([128, s_tot, 2], mybir.dt.float32)

            nc.sync.dma_start(src_t[:], src_d[:])
            nc.sync.dma_start(dst_t[:], dst_d[:])
            nc.sync.dma_start(w1_t[:], w1_d[:])
            nc.sync.dma_start(w2_t[:], w2_d[:])
            nc.sync.dma_start(w3_t[:], w3_d[:])
            nc.sync.dma_start(b1_t[:], b1_d[:])
            nc.sync.dma_start(b2_t[:], b2_d[:])
            nc.sync.dma_start(b3_t[:], b3_d[:])
            if SKIP_COMPUTE:
                nc.vector.memset(outb[:], 0.0)

            if SBUF_NODE:
                n_ranks = -(-n_nodes // 128)
                node_t = cpool.tile([128, n_ranks * H], mybir.dt.bfloat16)
                nc.sync.dma_start(node_t[:], node_d[:])
                node_lo = node_t[:, 0:min(SPLIT, n_ranks * H)]
                node_hi = (node_t[:, SPLIT:n_ranks * H]
                           if n_ranks * H > SPLIT else node_lo)
            else:
                node_lo = node_d[0:min(SPLIT, n_nodes), :]
                node_hi = (node_d[SPLIT:n_nodes, :]
                           if n_nodes > SPLIT else node_lo)
            g_src_base = [node_lo, node_lo, node_hi, node_hi]
            g_dst_base = [node_lo, node_hi, node_lo, node_hi]

            for _rep in range(reps):
                _emit_edge_loop(nc, tc, n_tiles_per_group, g_src_base,
                                g_dst_base, src_t, dst_t, w1_t, w2_t, w3_t,
                                b1_t, b2_t, b3_t, outb, wpool, mpool,
                                psA, psB, psC, psO, relu)

            nc.sync.dma_start(out_d[:], outb[:])

    nc.compile()
    return nc


def _emit_edge_loop(nc, tc, n_tiles_per_group, g_src_base, g_dst_base,
                    src_t, dst_t, w1_t, w2_t, w3_t, b1_t, b2_t, b3_t, outb,
                    wpool, mpool, psA, psB, psC, psO, relu):
    if True:
        if True:
            pos = 0  # running padded-edge offset (multiple of T_MM)
            blk_i = 0
            for g in range(4):
                n_t = n_tiles_per_group[g]
                if n_t == 0:
                    continue
                n_edges_g = n_t * T_MM
                # gather blocks of up to T_GATHER edges
                blocks = []
                off = 0
                while off < n_edges_g:
                    b = min(T_GATHER, n_edges_g - off)
                    blocks.append((pos + off, b))
                    off += b
                for (bpos, blen) in blocks:
                    gshape = ([128, T_GATHER // 128, H] if NT_GATHER
                              else [128, 1, T_GATHER])
                    hi = wpool.tile(gshape, mybir.dt.bfloat16, tag="hi")
                    hj = wpool.tile(gshape, mybir.dt.bfloat16, tag="hj")
                    i0 = bpos // 16
                    iw = blen // 16
                    gkw = dict(num_idxs=blen, num_idxs_reg=blen, elem_size=H,
                               transpose=True)
                    if NT_GATHER:
                        gkw["transpose"] = False
                    elif SBUF_NODE:
                        gkw.update(sbuf_tokens_per_rank=128,
                                   sbuf_free_dim_per_rank=2 * H,
                                   sbuf_free_dim_pad_per_rank=0,
                                   sbuf_byte_offset=0)
                    if SKIP_GATHER:
                        nc.gpsimd.memset(hi[:], 0)
                        nc.gpsimd.memset(hj[:], 0)
                    else:
                        qs = (2 * blk_i) % N_QUEUES
                        qd = (2 * blk_i + 1) % N_QUEUES
                        hi_ap = (hi[:, :blen // 128, :] if NT_GATHER
                                 else hi[:, :, :blen])
                        hj_ap = (hj[:, :blen // 128, :] if NT_GATHER
                                 else hj[:, :, :blen])
                        nc.gpsimd.dma_gather(
                            hi_ap, g_src_base[g],
                            src_t[:, i0:i0 + iw], queue_num=qs, **gkw)
                        nc.gpsimd.dma_gather(
                            hj_ap, g_dst_base[g],
                            dst_t[:, i0:i0 + iw], queue_num=qd, **gkw)
                    blk_i += 1
                    if SKIP_COMPUTE:
                        continue

                    for tt in range(blen // T_MM):
                        sl = slice(tt * T_MM, (tt + 1) * T_MM)
                        hiv = hi[:, 0, sl]
                        hjv = hj[:, 0, sl]
                        dv = mpool.tile([128, T_MM], mybir.dt.bfloat16,
                                        tag="dv")
                        pv = mpool.tile([128, T_MM], mybir.dt.bfloat16,
                                        tag="pv")
                        nc.vector.tensor_sub(dv[:], hiv, hjv)
                        # |x| on bf16 = clear the sign bit
                        dvu = dv[:].bitcast(mybir.dt.uint16)
                        nc.vector.tensor_scalar(
                            dvu, dvu, 0x7FFF, None,
                            mybir.AluOpType.bitwise_and)
                        nc.vector.tensor_mul(pv[:], hiv, hjv)

                        chunks = (hiv, hjv, dv[:], pv[:])
                        x1a = psA.tile([128, T_MM], mybir.dt.float32,
                                       tag="x1a", space="PSUM")
                        x1b = psB.tile([128, T_MM], mybir.dt.float32,
                                       tag="x1b", space="PSUM")
                        for m, x1p in enumerate((x1a, x1b)):
                            for kc in range(4):
                                nc.tensor.matmul(
                                    x1p[:],
                                    lhsT=w1_t[:, kc, m * H:(m + 1) * H],
                                    rhs=chunks[kc],
                                    start=(kc == 0), stop=(kc == 3))
                        x1s = []
                        for m, x1p in enumerate((x1a, x1b)):
                            x1 = mpool.tile([128, T_MM], mybir.dt.bfloat16,
                                            tag=f"x1_{m}")
                            nc.scalar.activation(x1[:], x1p[:], relu,
                                                 bias=b1_t[:, m:m + 1])
                            x1s.append(x1)

                        x2p = psC.tile([128, T_MM], mybir.dt.float32,
                                       tag="x2p", space="PSUM")
                        for m in range(2):
                            nc.tensor.matmul(x2p[:], lhsT=w2_t[:, m, :],
                                             rhs=x1s[m][:],
                                             start=(m == 0), stop=(m == 1))
                        x2 = mpool.tile([128, T_MM], mybir.dt.bfloat16,
                                        tag="x2")
                        nc.scalar.activation(x2[:], x2p[:], relu,
                                             bias=b2_t[:, 0:1])

                        # matmul PSUM writes must land at 512B-aligned offsets
                        # within a bank, so pad each [128,2] result to a
                        # 128-float row.
                        op = psO.tile([128, 4, 128], mybir.dt.float32,
                                      tag="op", space="PSUM")
                        for j in range(4):
                            nc.tensor.matmul(op[:, j, 0:2],
                                             lhsT=x2[:, j * H:(j + 1) * H],
                                             rhs=w3_t[:],
                                             start=True, stop=True)
                        s0 = (bpos + tt * T_MM) // H
                        nc.vector.tensor_add(outb[:, s0:s0 + 4, :],
                                             op[:, :, 0:2], b3_t[:])
                pos += n_edges_g


def _prep_core(src_c, dst_c, n_tiles_per_group, tot):
    """Bucket one core's edges; return (src16, dst16, slot_to_edge)."""
    bucket = (src_c >= SPLIT).astype(np.int8) * 2 + (dst_c >= SPLIT)
    if SORT_SRC:
        order = np.lexsort((src_c, bucket))
    else:
        order = np.argsort(bucket, kind="stable")
    src16 = np.zeros(tot, dtype=np.int16)
    dst16 = np.zeros(tot, dtype=np.int16)
    slot_to_edge = np.full(tot, -1, dtype=np.int64)
    pos = 0
    ostart = 0
    for g in range(4):
        cnt = int((bucket == g).sum())
        sel = order[ostart:ostart + cnt]
        ostart += cnt
        s = src_c[sel]
        d = dst_c[sel]
        if g >= 2:
            s = s - SPLIT
        if g % 2 == 1:
            d = d - SPLIT
        if not ZERO_IDX:
            src16[pos:pos + cnt] = s.astype(np.int16)
            dst16[pos:pos + cnt] = d.astype(np.int16)
        slot_to_edge[pos:pos + cnt] = sel
        pos += n_tiles_per_group[g] * T_MM
    return src16, dst16, slot_to_edge


def _wrap_idx(idx16):
    """[TOT] int16 -> [128, TOT/16] wrapped(16) + replicated(x8) layout."""
    w = idx16.reshape(-1, 16).T  # [16, TOT/16]
    return np.ascontiguousarray(np.tile(w, (8, 1)))


_PROGRAM_CACHE = {}


class _Runner:
    """Compile a Bass program once into a reusable sharded PJRT executable.

    Mirrors bass2jax.run_bass_via_pjrt but keeps the jitted callable (and the
    device-resident inputs) alive so repeat invocations only pay dispatch +
    execute, not NEFF re-load.
    """

    def __init__(self, nc):
        import jax
        from jax.sharding import Mesh, PartitionSpec
        from jax.experimental.shard_map import shard_map
        from concourse import bass2jax as b2j

        b2j.install_neuronx_cc_hook()
        self._jax = jax
        self.nc = nc

        in_names, out_names, out_avals, zero_outs = [], [], [], []
        partition_name = (nc.partition_id_tensor.name
                          if nc.partition_id_tensor else None)
        for alloc in nc.m.functions[0].allocations:
            if not isinstance(alloc, mybir.MemoryLocationSet):
                continue
            name = alloc.memorylocations[0].name
            if alloc.kind == "ExternalInput":
                if name != partition_name:
                    in_names.append(name)
            elif alloc.kind == "ExternalOutput":
                out_names.append(name)
                shape = tuple(alloc.tensor_shape)
                dtype = mybir.dt.np(alloc.dtype)
                out_avals.append(jax.core.ShapedArray(shape, dtype))
                zero_outs.append(np.zeros(shape, dtype))
        self.in_names = in_names
        self.out_names = out_names
        self.out_avals = out_avals
        self.zero_outs = zero_outs
        n_params = len(in_names)
        n_outs = len(out_names)
        all_in_names = list(in_names) + list(out_names)
        if partition_name is not None:
            all_in_names.append(partition_name)

        def _body(*args):
            operands = list(args)
            if partition_name is not None:
                operands.append(b2j.partition_id_tensor())
            outs = b2j._bass_exec_p.bind(
                *operands,
                out_avals=tuple(out_avals),
                in_names=tuple(all_in_names),
                out_names=tuple(out_names),
                lowering_input_output_aliases=(),
                sim_require_finite=True,
                sim_require_nnan=True,
                nc=nc,
            )
            return tuple(outs)

        devices = jax.devices()[:N_CORES]
        mesh = Mesh(np.asarray(devices), ("core",))
        in_specs = (PartitionSpec("core"),) * (n_params + n_outs)
        out_specs = (PartitionSpec("core"),) * n_outs
        donate = tuple(range(n_params, n_params + n_outs))
        self.sharded = jax.jit(
            shard_map(_body, mesh=mesh, in_specs=in_specs,
                      out_specs=out_specs, check_rep=False),
            donate_argnums=donate, keep_unused=True)
        self._resident = None

    def set_inputs(self, in_maps):
        jax = self._jax
        concat_in = [
            np.concatenate([np.asarray(in_maps[c][n])
                            for c in range(N_CORES)], axis=0)
            for n in self.in_names
        ]
        self._resident = [jax.device_put(a) for a in concat_in]
        jax.block_until_ready(self._resident)

    def execute(self):
        jax = self._jax
        zeros = [np.zeros((N_CORES * z.shape[0], *z.shape[1:]), z.dtype)
                 for z in self.zero_outs]
        out_arrs = self.sharded(*self._resident, *zeros)
        jax.block_until_ready(out_arrs)
        return out_arrs

    def run(self):
        out_arrs = self.execute()
        return [
            {name: np.asarray(out_arrs[i]).reshape(
                N_CORES, *self.out_avals[i].shape)[c]
             for i, name in enumerate(self.out_names)}
            for c in range(N_CORES)
        ]


def kernel(node_repr, src, dst, W1, b1, W2, b2, W3, b3, _reps=1):
    node_repr = np.asarray(node_repr, dtype=np.float32)
    src = np.asarray(src).astype(np.int64)
    dst = np.asarray(dst).astype(np.int64)
    W1 = np.asarray(W1, dtype=np.float32)
    b1 = np.asarray(b1, dtype=np.float32)
    W2 = np.asarray(W2, dtype=np.float32)
    b2 = np.asarray(b2, dtype=np.float32)
    W3 = np.asarray(W3, dtype=np.float32)
    b3 = np.asarray(b3, dtype=np.float32)

    n_nodes = node_repr.shape[0]
    E = src.shape[0]
    assert node_repr.shape[1] == H
    assert E % N_CORES == 0
    e_core = E // N_CORES

    # ---- host-side sharding + bucketing ----
    src_pc = src.reshape(N_CORES, e_core)
    dst_pc = dst.reshape(N_CORES, e_core)
    buckets = (src_pc >= SPLIT).astype(np.int8) * 2 + (dst_pc >= SPLIT)
    counts = np.stack([(buckets == g).sum(axis=1) for g in range(4)], axis=1)
    n_tiles_per_group = tuple(
        int(-(-counts[:, g].max() // T_MM)) for g in range(4))
    tot = sum(n_tiles_per_group) * T_MM

    node_bf = node_repr.astype(BF16)
    if SBUF_NODE:
        n_ranks = -(-n_nodes // 128)
        pad = np.zeros((n_ranks * 128, H), dtype=BF16)
        pad[:n_nodes] = node_bf
        node_bf = np.ascontiguousarray(
            pad.reshape(n_ranks, 128, H).transpose(1, 0, 2)
            .reshape(128, n_ranks * H))
    w1_dev = np.ascontiguousarray(
        W1.astype(BF16).reshape(4, 128, 2 * H).transpose(1, 0, 2))
    w2_dev = np.ascontiguousarray(
        W2.astype(BF16).reshape(2, 128, H).transpose(1, 0, 2))
    w3_dev = np.ascontiguousarray(W3.astype(BF16))
    b1_dev = np.ascontiguousarray(b1.reshape(2, 128).T.astype(np.float32))
    b2_dev = b2.reshape(128, 1).astype(np.float32)
    b3_dev = np.ascontiguousarray(
        np.tile(b3.astype(np.float32), (128, 4)).reshape(128, 4, 2))

    in_maps = []
    slot_maps = []
    for c in range(N_CORES):
        s16, d16, s2e = _prep_core(src_pc[c], dst_pc[c],
                                   n_tiles_per_group, tot)
        slot_maps.append(s2e)
        in_maps.append({
            "node": node_bf,
            "srcidx": _wrap_idx(s16),
            "dstidx": _wrap_idx(d16),
            "w1": w1_dev, "w2": w2_dev, "w3": w3_dev,
            "b1": b1_dev, "b2": b2_dev, "b3": b3_dev,
        })

    # ---- build + compile (cached per tile-count signature) ----
    key = (n_tiles_per_group, n_nodes, tot, _reps)
    runner = _PROGRAM_CACHE.get(key)
    if runner is None:
        nc = _build_program(n_tiles_per_group, n_nodes, tot, reps=_reps)
        runner = _Runner(nc)
        _PROGRAM_CACHE[key] = runner
    runner.set_inputs(in_maps)
    results = runner.run()

    # ---- gather / unpermute ----
    out = np.empty((E, 2), dtype=np.float32)
    for c in range(N_CORES):
        rows = results[c]["out"].transpose(1, 0, 2).reshape(-1, 2)
        s2e = slot_maps[c]
        valid = s2e >= 0
        out[c * e_core + s2e[valid]] = rows[valid]

    global _LAST_RUNNER
    _LAST_RUNNER = runner
    return out


_LAST_RUNNER = None
